# revision 8
# baseline (speedup 1.0000x reference)
"""GraphSelfAttentionLayer Trainium2 kernel.

Problem: B,N,F,H = 8,1024,1024,8 (HD=128). Data-parallel over B across the
8 NeuronCores (one batch element per core, weights replicated; no
collectives). Per core:

    q = obj @ Wq.T * 1/sqrt(HD)   (scale folded into Wq host-side)
    k = cross @ Wk.T ; v = cross @ Wv.T
    att_h = q_h @ k_h.T                      (per head, HD=128)
    A_u_h = exp(att_h) * expM                (expM = (adj>0)*exp(label_bias),
                                              host-precomputed multiplicative
                                              mask -- no -9e15 on device)
    S_h   = rowsum(A_u_h)  ; rs_h = 1/S_h
    out_h = (A_u_h @ (v @ Wo_h.T + bo_h)) * rs_h    (algebraic fusion:
                                              (A@v)@Wo.T == A@(v@Wo.T))
    att_avg = sum_h A_u_h * rs_h / H

All matmuls run in bf16 (fp32 PSUM accumulation). Softmax skips the rowmax
subtraction: scores are ~N(0, 0.41) so exp() is safely in range, and masked
entries are exact zeros via expM. Normalization is deferred past the AV
matmul. All layout transposes (activations in, unnormalized attention A_u,
per-head output) ride the DMA XBAR transpose (2-byte dtype) instead of the
TensorEngine, keeping the PE for real matmuls. att_avg accumulation runs on
GPSIMD (Pool), which is otherwise idle.
"""

import sys

sys.path.insert(0, "/opt/trn_rl_repo")

import contextlib

import numpy as np
import ml_dtypes

import concourse.bass as bass
import concourse.tile as tile
from concourse import bacc, mybir
from concourse.bass_utils import run_bass_kernel_spmd

BF16 = mybir.dt.bfloat16
F32 = mybir.dt.float32
AF = mybir.ActivationFunctionType
ALU = mybir.AluOpType

P = 128
B, N, F, H = 8, 1024, 1024, 8
HD = F // H  # 128
CH = F // P  # 8 feature chunks
NCH = N // P  # 8 row chunks
NH = N // 512  # 2 free-dim halves

_PROG = None  # cached compiled Bass program (built for zero biases or not)


def _build_program(time_reps=1, with_bias=True):
    """time_reps>1 wraps the body in a hardware loop so marginal wall-clock
    per iteration isolates true NEFF execution time from the remote-dispatch
    floor. with_bias=False drops the per-partition bias adds (all-zero
    biases) so projection PSUM->SBUF copies can balance across engines."""
    nc = bacc.Bacc("TRN2", target_bir_lowering=False, debug=False, num_devices=8)

    obj_d = nc.dram_tensor("obj", [N, F], F32, kind="ExternalInput")
    cross_d = nc.dram_tensor("cross", [N, F], F32, kind="ExternalInput")
    expm_d = nc.dram_tensor("expm", [N, N], BF16, kind="ExternalInput")
    wqt_d = nc.dram_tensor("wqt", [F, F], BF16, kind="ExternalInput")
    wkt_d = nc.dram_tensor("wkt", [F, F], BF16, kind="ExternalInput")
    wvt_d = nc.dram_tensor("wvt", [F, F], BF16, kind="ExternalInput")
    wot_d = nc.dram_tensor("wot", [F, F], BF16, kind="ExternalInput")
    bq_d = nc.dram_tensor("bq", [F], F32, kind="ExternalInput")
    bk_d = nc.dram_tensor("bk", [F], F32, kind="ExternalInput")
    bv_d = nc.dram_tensor("bv", [F], F32, kind="ExternalInput")
    bo_rep_d = nc.dram_tensor("bo_rep", [P, F], F32, kind="ExternalInput")
    out_d = nc.dram_tensor("out", [N, F], F32, kind="ExternalOutput")
    avg_d = nc.dram_tensor("att_avg", [N, N], F32, kind="ExternalOutput")

    with tile.TileContext(nc) as tc:
        with (
            tc.For_i(0, time_reps, 1) if time_reps > 1 else contextlib.nullcontext(),
            tc.tile_pool(name="persist", bufs=1) as persist,
            tc.tile_pool(name="big", bufs=4) as big,
            tc.tile_pool(name="stage", bufs=3) as stage,
            tc.tile_pool(name="small", bufs=3) as small,
        ):
            qT = persist.tile([P, CH, N], BF16, tag="qT")
            kT = persist.tile([P, CH, N], BF16, tag="kT")
            vW = persist.tile([P, CH, F], BF16, tag="vW")
            expM = persist.tile([P, NCH, N], BF16, tag="expM")
            acc = persist.tile([P, NCH, N], BF16, tag="acc")
            bo_rep = persist.tile([P, F], F32, tag="bo_rep")

            nc.sync.dma_start(bo_rep[:], bo_rep_d[:])
            nc.sync.dma_start(expM[:], expm_d.ap().rearrange("(no p) m -> p no m", p=P))
            if with_bias:
                bq_t = persist.tile([P, CH], F32, tag="bq")
                bk_t = persist.tile([P, CH], F32, tag="bk")
                bv_t = persist.tile([P, CH], F32, tag="bv")
                nc.sync.dma_start(bq_t[:], bq_d.ap().rearrange("(o p) -> p o", p=P))
                nc.sync.dma_start(bk_t[:], bk_d.ap().rearrange("(o p) -> p o", p=P))
                nc.sync.dma_start(bv_t[:], bv_d.ap().rearrange("(o p) -> p o", p=P))

            def load_w(dram):
                w = big.tile([P, CH, F], BF16, tag="big")
                nc.sync.dma_start(w[:], dram.ap().rearrange("(co p) f -> p co f", p=P))
                return w

            # ---- Phase A: input transposes (XBAR), QKV projections, vW ----
            with tc.tile_pool(name="psA", bufs=3, space="PSUM") as psA:

                def transpose_in(x_dram):
                    """[N, F] f32 DRAM -> [P, CH, N] bf16 SBUF feature-major:
                    stage rows, convert to bf16, DMA-XBAR transpose."""
                    xT = big.tile([P, CH, N], BF16, tag="big")
                    for no in range(NCH):
                        stg = stage.tile([P, F], F32, tag="stg")
                        nc.sync.dma_start(stg[:], x_dram.ap()[no * P : (no + 1) * P, :])
                        cvt = stage.tile([P, F], BF16, tag="cvt")
                        nc.gpsimd.tensor_copy(cvt[:], stg[:])
                        nc.sync.dma_start_transpose(
                            xT[:, :, no * P : (no + 1) * P], cvt[:]
                        )
                    return xT

                def project(dst, wT, srcT, bias_t):
                    for fo in range(CH):
                        for nh in range(NH):
                            ps = psA.tile([P, 512], F32, tag="psA")
                            for co in range(CH):
                                nc.tensor.matmul(
                                    ps[:],
                                    lhsT=wT[:, co, fo * P : (fo + 1) * P],
                                    rhs=srcT[:, co, nh * 512 : (nh + 1) * 512],
                                    start=(co == 0),
                                    stop=(co == CH - 1),
                                )
                            dslc = dst[:, fo, nh * 512 : (nh + 1) * 512]
                            if with_bias:
                                nc.scalar.activation(
                                    dslc,
                                    ps[:],
                                    AF.Identity,
                                    bias=bias_t[:, fo : fo + 1],
                                )
                            else:
                                nc.any.tensor_copy(dslc, ps[:])

                wk = load_w(wkt_d)
                wv = load_w(wvt_d)
                crossT = transpose_in(cross_d)
                project(kT, wk, crossT, bk_t if with_bias else None)
                vT = big.tile([P, CH, N], BF16, tag="big")
                project(vT, wv, crossT, bv_t if with_bias else None)

                # vW[m, f'] = sum_f vT[f,m] * WoT[f,f'] + bo[f']
                wo = load_w(wot_d)
                for mo in range(CH):
                    for fh in range(NH):
                        ps = psA.tile([P, 512], F32, tag="psA")
                        for fo in range(CH):
                            nc.tensor.matmul(
                                ps[:],
                                lhsT=vT[:, fo, mo * P : (mo + 1) * P],
                                rhs=wo[:, fo, fh * 512 : (fh + 1) * 512],
                                start=(fo == 0),
                                stop=(fo == CH - 1),
                            )
                        if with_bias:
                            nc.vector.tensor_add(
                                vW[:, mo, fh * 512 : (fh + 1) * 512],
                                ps[:],
                                bo_rep[:, fh * 512 : (fh + 1) * 512],
                            )
                        else:
                            nc.any.tensor_copy(
                                vW[:, mo, fh * 512 : (fh + 1) * 512], ps[:]
                            )

                wq = load_w(wqt_d)
                objT = transpose_in(obj_d)
                project(qT, wq, objT, bq_t if with_bias else None)

            # ---- Phase B: per-head attention (software-pipelined) ----
            with (
                tc.tile_pool(name="psatt", bufs=3, space="PSUM") as psatt,
                tc.tile_pool(name="psav", bufs=2, space="PSUM") as psav,
            ):
                st = {}  # per-head stage-1 products

                def stage1(h):
                    A_u = big.tile([P, NCH, N], BF16, tag="big")
                    S = small.tile([P, NCH], F32, tag="S")
                    for no in range(NCH):
                        pa = psatt.tile([P, N], F32, tag="att")
                        for mh in range(NH):
                            nc.tensor.matmul(
                                pa[:, mh * 512 : (mh + 1) * 512],
                                lhsT=qT[:, h, no * P : (no + 1) * P],
                                rhs=kT[:, h, mh * 512 : (mh + 1) * 512],
                                start=True,
                                stop=True,
                            )
                        ex = stage.tile([P, N], BF16, tag="exp")
                        nc.scalar.activation(ex[:], pa[:], AF.Exp)
                        nc.vector.scalar_tensor_tensor(
                            out=A_u[:, no, :],
                            in0=ex[:],
                            scalar=1.0,
                            in1=expM[:, no, :],
                            op0=ALU.mult,
                            op1=ALU.mult,
                            accum_out=S[:, no : no + 1],
                        )
                    rs = small.tile([P, NCH], F32, tag="rs")
                    rs8 = small.tile([P, NCH], F32, tag="rs8")
                    nc.vector.reciprocal(rs[:], S[:])
                    nc.vector.tensor_scalar_mul(rs8[:], rs[:], 1.0 / H)
                    st[h] = (A_u, rs, rs8)

                def stage2(h):
                    A_u, rs, rs8 = st.pop(h)
                    # transpose A_u via DMA XBAR: A_uT[p,mo,n] = A_u[n, mo*128+p]
                    A_uT = big.tile([P, CH, N], BF16, tag="big")
                    for no in range(NCH):
                        nc.sync.dma_start_transpose(
                            A_uT[:, :, no * P : (no + 1) * P], A_u[:, no, :]
                        )
                    # outT[hd, n] = sum_m vW[m, h*HD+hd] * A_uT[m, n]
                    outT = stage.tile([P, N], BF16, tag="outT")
                    for ng in range(NH):
                        pav = psav.tile([P, 512], F32, tag="av")
                        for mo in range(CH):
                            nc.tensor.matmul(
                                pav[:],
                                lhsT=vW[:, mo, h * HD : (h + 1) * HD],
                                rhs=A_uT[:, mo, ng * 512 : (ng + 1) * 512],
                                start=(mo == 0),
                                stop=(mo == CH - 1),
                            )
                        nc.any.tensor_copy(outT[:, ng * 512 : (ng + 1) * 512], pav[:])
                    # back to row-major: outN[p, no, hd] = outT[hd, no*128+p]
                    outN = stage.tile([P, NCH, HD], BF16, tag="outN")
                    nc.sync.dma_start_transpose(outN[:], outT[:])
                    for no in range(NCH):
                        ot = small.tile([P, HD], F32, tag="ot")
                        nc.vector.tensor_scalar_mul(
                            ot[:], outN[:, no, :], rs[:, no : no + 1]
                        )
                        nc.sync.dma_start(
                            out_d.ap()[no * P : (no + 1) * P, h * HD : (h + 1) * HD],
                            ot[:],
                        )
                    # att_avg accumulation (walrus rejects TensorScalarPtr on
                    # Pool, so this stays on DVE)
                    for no in range(NCH):
                        if h == 0:
                            nc.vector.tensor_scalar_mul(
                                acc[:, no, :], A_u[:, no, :], rs8[:, no : no + 1]
                            )
                        else:
                            nc.vector.scalar_tensor_tensor(
                                out=acc[:, no, :],
                                in0=A_u[:, no, :],
                                scalar=rs8[:, no : no + 1],
                                in1=acc[:, no, :],
                                op0=ALU.mult,
                                op1=ALU.add,
                            )

                for h in range(H):
                    stage1(h)
                    if h > 0:
                        stage2(h - 1)
                stage2(H - 1)

            # ---- Phase C: att_avg convert + out ----
            for no in range(NCH):
                cv = stage.tile([P, N], F32, tag="cvf")
                nc.gpsimd.tensor_copy(cv[:], acc[:, no, :])
                nc.sync.dma_start(avg_d.ap()[no * P : (no + 1) * P, :], cv[:])

    nc.compile()
    return nc


def _get_program(with_bias=True):
    global _PROG
    if _PROG is None or _PROG[1] != with_bias:
        _PROG = (_build_program(with_bias=with_bias), with_bias)
    return _PROG[0]


def kernel(
    obj_feats,
    cross_feats,
    adj_matrix,
    label_biases_att,
    Wq,
    bq,
    Wk,
    bk,
    Wv,
    bv,
    Wo,
    bo,
):
    obj_feats = np.asarray(obj_feats, np.float32)
    cross_feats = np.asarray(cross_feats, np.float32)
    adj_matrix = np.asarray(adj_matrix)
    label_biases_att = np.asarray(label_biases_att, np.float32)
    Wq = np.asarray(Wq, np.float32)
    Wk = np.asarray(Wk, np.float32)
    Wv = np.asarray(Wv, np.float32)
    Wo = np.asarray(Wo, np.float32)
    bq = np.asarray(bq, np.float32)
    bk = np.asarray(bk, np.float32)
    bv = np.asarray(bv, np.float32)
    bo = np.asarray(bo, np.float32)

    bf16 = ml_dtypes.bfloat16
    s = np.float32(1.0 / np.sqrt(HD))
    wqt = np.ascontiguousarray((Wq.T * s).astype(bf16))  # [C, F], scale folded
    wkt = np.ascontiguousarray(Wk.T.astype(bf16))
    wvt = np.ascontiguousarray(Wv.T.astype(bf16))
    # WoT[f, h*HD+hd] = Wo[h, hd, f]
    wot = np.ascontiguousarray(Wo.transpose(2, 0, 1).reshape(F, F).astype(bf16))
    bo_rep = np.ascontiguousarray(np.broadcast_to(bo, (P, F)).astype(np.float32))
    bq_s = (bq * s).astype(np.float32)

    with_bias = bool(np.any(bq) or np.any(bk) or np.any(bv) or np.any(bo))

    # multiplicative mask: exp(label_bias) where adj>0 else 0
    expm = np.where(adj_matrix > 0, np.exp(label_biases_att), np.float32(0.0)).astype(
        bf16
    )

    nc = _get_program(with_bias=with_bias)
    in_maps = []
    for b in range(B):
        in_maps.append(
            {
                "obj": np.ascontiguousarray(obj_feats[b]),
                "cross": np.ascontiguousarray(cross_feats[b]),
                "expm": np.ascontiguousarray(expm[b]),
                "wqt": wqt,
                "wkt": wkt,
                "wvt": wvt,
                "wot": wot,
                "bq": bq_s,
                "bk": bk,
                "bv": bv,
                "bo_rep": bo_rep,
            }
        )
    res = run_bass_kernel_spmd(nc, in_maps, core_ids=list(range(B)))
    out = np.stack([res.results[b]["out"] for b in range(B)])
    att_avg = np.stack([res.results[b]["att_avg"] for b in range(B)])
    return out, att_avg


# revision 15
# speedup vs baseline: 1.4662x; 1.4662x over previous
"""GraphSelfAttentionLayer Trainium2 kernel.

Problem: B,N,F,H = 8,1024,1024,8 (HD=128). Data-parallel over B across the
8 NeuronCores (one batch element per core, weights replicated; no
collectives). Per core:

    q = obj @ Wq.T * 1/sqrt(HD)   (scale folded into Wq host-side)
    k = cross @ Wk.T ; v = cross @ Wv.T
    att_h = q_h @ k_h.T                      (per head, HD=128)
    A_u_h = exp(att_h + M)                   (M = label_bias + (adj-1)*9e15,
                                              host-precomputed additive mask,
                                              injected into PSUM by an extra
                                              identity-stationary matmul)
    S_h   = rowsum(A_u_h)                    (free via ACT accum_out)
    rs_h  = 1/S_h
    out_h = (A_u_h @ (v @ Wo_h.T + bo_h)) * rs_h    (algebraic fusion:
                                              (A@v)@Wo.T == A@(v@Wo.T))
    att_avg = sum_h A_u_h * rs_h / H

All matmuls run in bf16 (fp32 PSUM accumulation). Softmax skips the rowmax
subtraction: scores are ~N(0, 0.41) so exp() is safely in range, and masked
entries are exact zeros via expM. Normalization is deferred past the AV
matmul. All layout transposes (activations in, unnormalized attention A_u,
per-head output) ride the DMA XBAR transpose (2-byte dtype) instead of the
TensorEngine, keeping the PE for real matmuls. att_avg accumulation runs on
GPSIMD (Pool), which is otherwise idle.
"""

import sys

sys.path.insert(0, "/opt/trn_rl_repo")

import contextlib

import numpy as np
import ml_dtypes

import concourse.bass as bass
import concourse.tile as tile
from concourse import bacc, mybir
from concourse.bass_utils import run_bass_kernel_spmd
from concourse.masks import make_identity

BF16 = mybir.dt.bfloat16
F32 = mybir.dt.float32
AF = mybir.ActivationFunctionType
ALU = mybir.AluOpType

P = 128
B, N, F, H = 8, 1024, 1024, 8
HD = F // H  # 128
CH = F // P  # 8 feature chunks
NCH = N // P  # 8 row chunks
NH = N // 512  # 2 free-dim halves

_PROG = None  # cached compiled Bass program (built for zero biases or not)


def _build_program(time_reps=1, with_bias=True):
    """time_reps>1 wraps the body in a hardware loop so marginal wall-clock
    per iteration isolates true NEFF execution time from the remote-dispatch
    floor. with_bias=False drops the per-partition bias adds (all-zero
    biases) so projection PSUM->SBUF copies can balance across engines."""
    nc = bacc.Bacc("TRN2", target_bir_lowering=False, debug=False, num_devices=8)

    obj_d = nc.dram_tensor("obj", [N, F], F32, kind="ExternalInput")
    cross_d = nc.dram_tensor("cross", [N, F], F32, kind="ExternalInput")
    mcomb_d = nc.dram_tensor("mcomb", [N, N], BF16, kind="ExternalInput")
    wqt_d = nc.dram_tensor("wqt", [F, F], BF16, kind="ExternalInput")
    wkt_d = nc.dram_tensor("wkt", [F, F], BF16, kind="ExternalInput")
    wvt_d = nc.dram_tensor("wvt", [F, F], BF16, kind="ExternalInput")
    wot_d = nc.dram_tensor("wot", [F, F], BF16, kind="ExternalInput")
    bq_d = nc.dram_tensor("bq", [F], F32, kind="ExternalInput")
    bk_d = nc.dram_tensor("bk", [F], F32, kind="ExternalInput")
    bv_d = nc.dram_tensor("bv", [F], F32, kind="ExternalInput")
    bo_rep_d = nc.dram_tensor("bo_rep", [P, F], F32, kind="ExternalInput")
    out_d = nc.dram_tensor("out", [N, F], F32, kind="ExternalOutput")
    avg_d = nc.dram_tensor("att_avg", [N, N], F32, kind="ExternalOutput")

    with tile.TileContext(nc) as tc:
        with (
            tc.For_i(0, time_reps, 1) if time_reps > 1 else contextlib.nullcontext(),
            tc.tile_pool(name="persist", bufs=1) as persist,
            tc.tile_pool(name="big", bufs=4) as big,
            tc.tile_pool(name="stage", bufs=3) as stage,
            tc.tile_pool(name="small", bufs=3) as small,
        ):
            qT = persist.tile([P, CH, N], BF16, tag="qT")
            kT = persist.tile([P, CH, N], BF16, tag="kT")
            vW = persist.tile([P, CH, F], BF16, tag="vW")
            mcomb = persist.tile([P, NCH, N], BF16, tag="mcomb")
            acc = persist.tile([P, NCH, N], BF16, tag="acc")
            bo_rep = persist.tile([P, F], F32, tag="bo_rep")
            ident = persist.tile([P, P], BF16, tag="ident")
            make_identity(nc, ident[:])

            nc.sync.dma_start(bo_rep[:], bo_rep_d[:])
            nc.sync.dma_start(
                mcomb[:], mcomb_d.ap().rearrange("(no p) m -> p no m", p=P)
            )
            if with_bias:
                bq_t = persist.tile([P, CH], F32, tag="bq")
                bk_t = persist.tile([P, CH], F32, tag="bk")
                bv_t = persist.tile([P, CH], F32, tag="bv")
                nc.sync.dma_start(bq_t[:], bq_d.ap().rearrange("(o p) -> p o", p=P))
                nc.sync.dma_start(bk_t[:], bk_d.ap().rearrange("(o p) -> p o", p=P))
                nc.sync.dma_start(bv_t[:], bv_d.ap().rearrange("(o p) -> p o", p=P))

            def load_w(dram):
                w = big.tile([P, CH, F], BF16, tag="big")
                nc.sync.dma_start(w[:], dram.ap().rearrange("(co p) f -> p co f", p=P))
                return w

            # ---- Phase A: input transposes (XBAR), QKV projections, vW ----
            with tc.tile_pool(name="psA", bufs=3, space="PSUM") as psA:

                def transpose_in(x_dram):
                    """[N, F] f32 DRAM -> [P, CH, N] bf16 SBUF feature-major:
                    stage rows, convert to bf16, DMA-XBAR transpose."""
                    xT = big.tile([P, CH, N], BF16, tag="big")
                    for no in range(NCH):
                        stg = stage.tile([P, F], F32, tag="stg")
                        nc.sync.dma_start(stg[:], x_dram.ap()[no * P : (no + 1) * P, :])
                        cvt = stage.tile([P, F], BF16, tag="cvt")
                        nc.gpsimd.tensor_copy(cvt[:], stg[:])
                        nc.sync.dma_start_transpose(
                            xT[:, :, no * P : (no + 1) * P], cvt[:]
                        )
                    return xT

                def project(dst, wT, srcT, bias_t):
                    for fo in range(CH):
                        for nh in range(NH):
                            ps = psA.tile([P, 512], F32, tag="psA")
                            for co in range(CH):
                                nc.tensor.matmul(
                                    ps[:],
                                    lhsT=wT[:, co, fo * P : (fo + 1) * P],
                                    rhs=srcT[:, co, nh * 512 : (nh + 1) * 512],
                                    start=(co == 0),
                                    stop=(co == CH - 1),
                                )
                            dslc = dst[:, fo, nh * 512 : (nh + 1) * 512]
                            if with_bias:
                                nc.scalar.activation(
                                    dslc,
                                    ps[:],
                                    AF.Identity,
                                    bias=bias_t[:, fo : fo + 1],
                                )
                            else:
                                nc.any.tensor_copy(dslc, ps[:])

                wk = load_w(wkt_d)
                wv = load_w(wvt_d)
                crossT = transpose_in(cross_d)
                project(kT, wk, crossT, bk_t if with_bias else None)
                vT = big.tile([P, CH, N], BF16, tag="big")
                project(vT, wv, crossT, bv_t if with_bias else None)

                # vW[m, f'] = sum_f vT[f,m] * WoT[f,f'] + bo[f']
                wo = load_w(wot_d)
                for mo in range(CH):
                    for fh in range(NH):
                        ps = psA.tile([P, 512], F32, tag="psA")
                        for fo in range(CH):
                            nc.tensor.matmul(
                                ps[:],
                                lhsT=vT[:, fo, mo * P : (mo + 1) * P],
                                rhs=wo[:, fo, fh * 512 : (fh + 1) * 512],
                                start=(fo == 0),
                                stop=(fo == CH - 1),
                            )
                        if with_bias:
                            nc.vector.tensor_add(
                                vW[:, mo, fh * 512 : (fh + 1) * 512],
                                ps[:],
                                bo_rep[:, fh * 512 : (fh + 1) * 512],
                            )
                        else:
                            nc.any.tensor_copy(
                                vW[:, mo, fh * 512 : (fh + 1) * 512], ps[:]
                            )

                wq = load_w(wqt_d)
                objT = transpose_in(obj_d)
                project(qT, wq, objT, bq_t if with_bias else None)

            # ---- Phase B: per-head attention (software-pipelined) ----
            with (
                tc.tile_pool(name="psatt", bufs=3, space="PSUM") as psatt,
                tc.tile_pool(name="psav", bufs=2, space="PSUM") as psav,
            ):
                st = {}  # per-head stage-1 products

                def stage1(h):
                    A_u = big.tile([P, NCH, N], BF16, tag="big")
                    S = small.tile([P, NCH], F32, tag="S")
                    for no in range(NCH):
                        pa = psatt.tile([P, N], F32, tag="att")
                        for mh in range(NH):
                            nc.tensor.matmul(
                                pa[:, mh * 512 : (mh + 1) * 512],
                                lhsT=qT[:, h, no * P : (no + 1) * P],
                                rhs=kT[:, h, mh * 512 : (mh + 1) * 512],
                                start=True,
                                stop=False,
                            )
                            # additive mask via identity-stationary matmul:
                            # psum += I.T @ mcomb = mcomb
                            nc.tensor.matmul(
                                pa[:, mh * 512 : (mh + 1) * 512],
                                lhsT=ident[:],
                                rhs=mcomb[:, no, mh * 512 : (mh + 1) * 512],
                                start=False,
                                stop=True,
                            )
                        # masked exp + row sums in one ACT pass
                        nc.scalar.activation(
                            A_u[:, no, :], pa[:], AF.Exp, accum_out=S[:, no : no + 1]
                        )
                    rs = small.tile([P, NCH], F32, tag="rs")
                    rs8 = small.tile([P, NCH], F32, tag="rs8")
                    nc.vector.reciprocal(rs[:], S[:])
                    nc.vector.tensor_scalar_mul(rs8[:], rs[:], 1.0 / H)
                    st[h] = (A_u, rs, rs8)

                def stage2(h):
                    A_u, rs, rs8 = st.pop(h)
                    # transpose A_u via DMA XBAR: A_uT[p,mo,n] = A_u[n, mo*128+p]
                    A_uT = big.tile([P, CH, N], BF16, tag="big")
                    for no in range(NCH):
                        nc.sync.dma_start_transpose(
                            A_uT[:, :, no * P : (no + 1) * P], A_u[:, no, :]
                        )
                    # outT[hd, n] = sum_m vW[m, h*HD+hd] * A_uT[m, n]
                    outT = stage.tile([P, N], BF16, tag="outT")
                    for ng in range(NH):
                        pav = psav.tile([P, 512], F32, tag="av")
                        for mo in range(CH):
                            nc.tensor.matmul(
                                pav[:],
                                lhsT=vW[:, mo, h * HD : (h + 1) * HD],
                                rhs=A_uT[:, mo, ng * 512 : (ng + 1) * 512],
                                start=(mo == 0),
                                stop=(mo == CH - 1),
                            )
                        nc.any.tensor_copy(outT[:, ng * 512 : (ng + 1) * 512], pav[:])
                    # back to row-major: outN[p, no, hd] = outT[hd, no*128+p]
                    outN = stage.tile([P, NCH, HD], BF16, tag="outN")
                    nc.sync.dma_start_transpose(outN[:], outT[:])
                    for no in range(NCH):
                        ot = small.tile([P, HD], F32, tag="ot")
                        nc.vector.tensor_scalar_mul(
                            ot[:], outN[:, no, :], rs[:, no : no + 1]
                        )
                        nc.sync.dma_start(
                            out_d.ap()[no * P : (no + 1) * P, h * HD : (h + 1) * HD],
                            ot[:],
                        )
                    # att_avg accumulation (walrus rejects TensorScalarPtr on
                    # Pool, so this stays on DVE)
                    for no in range(NCH):
                        if h == 0:
                            nc.vector.tensor_scalar_mul(
                                acc[:, no, :], A_u[:, no, :], rs8[:, no : no + 1]
                            )
                        else:
                            nc.vector.scalar_tensor_tensor(
                                out=acc[:, no, :],
                                in0=A_u[:, no, :],
                                scalar=rs8[:, no : no + 1],
                                in1=acc[:, no, :],
                                op0=ALU.mult,
                                op1=ALU.add,
                            )

                for h in range(H):
                    stage1(h)
                    if h > 0:
                        stage2(h - 1)
                stage2(H - 1)

            # ---- Phase C: att_avg convert + out ----
            for no in range(NCH):
                cv = stage.tile([P, N], F32, tag="cvf")
                nc.gpsimd.tensor_copy(cv[:], acc[:, no, :])
                nc.sync.dma_start(avg_d.ap()[no * P : (no + 1) * P, :], cv[:])

    nc.compile()
    return nc


def _get_program(with_bias=True):
    global _PROG
    if _PROG is None or _PROG[1] != with_bias:
        _PROG = (_build_program(with_bias=with_bias), with_bias)
    return _PROG[0]


def kernel(
    obj_feats,
    cross_feats,
    adj_matrix,
    label_biases_att,
    Wq,
    bq,
    Wk,
    bk,
    Wv,
    bv,
    Wo,
    bo,
):
    obj_feats = np.asarray(obj_feats, np.float32)
    cross_feats = np.asarray(cross_feats, np.float32)
    adj_matrix = np.asarray(adj_matrix)
    label_biases_att = np.asarray(label_biases_att, np.float32)
    Wq = np.asarray(Wq, np.float32)
    Wk = np.asarray(Wk, np.float32)
    Wv = np.asarray(Wv, np.float32)
    Wo = np.asarray(Wo, np.float32)
    bq = np.asarray(bq, np.float32)
    bk = np.asarray(bk, np.float32)
    bv = np.asarray(bv, np.float32)
    bo = np.asarray(bo, np.float32)

    bf16 = ml_dtypes.bfloat16
    s = np.float32(1.0 / np.sqrt(HD))
    wqt = np.ascontiguousarray((Wq.T * s).astype(bf16))  # [C, F], scale folded
    wkt = np.ascontiguousarray(Wk.T.astype(bf16))
    wvt = np.ascontiguousarray(Wv.T.astype(bf16))
    # WoT[f, h*HD+hd] = Wo[h, hd, f]
    wot = np.ascontiguousarray(Wo.transpose(2, 0, 1).reshape(F, F).astype(bf16))
    bo_rep = np.ascontiguousarray(np.broadcast_to(bo, (P, F)).astype(np.float32))
    bq_s = (bq * s).astype(np.float32)

    with_bias = bool(np.any(bq) or np.any(bk) or np.any(bv) or np.any(bo))

    # additive mask: label_bias where adj>0 else -9e15 (exp underflows to 0)
    mcomb = np.where(
        adj_matrix > 0, label_biases_att, np.float32(-9e15) + label_biases_att
    ).astype(bf16)

    nc = _get_program(with_bias=with_bias)
    in_maps = []
    for b in range(B):
        in_maps.append(
            {
                "obj": np.ascontiguousarray(obj_feats[b]),
                "cross": np.ascontiguousarray(cross_feats[b]),
                "mcomb": np.ascontiguousarray(mcomb[b]),
                "wqt": wqt,
                "wkt": wkt,
                "wvt": wvt,
                "wot": wot,
                "bq": bq_s,
                "bk": bk,
                "bv": bv,
                "bo_rep": bo_rep,
            }
        )
    res = run_bass_kernel_spmd(nc, in_maps, core_ids=list(range(B)))
    out = np.stack([res.results[b]["out"] for b in range(B)])
    att_avg = np.stack([res.results[b]["att_avg"] for b in range(B)])
    return out, att_avg


# revision 18
# speedup vs baseline: 1.7744x; 1.2102x over previous
"""GraphSelfAttentionLayer Trainium2 kernel.

Problem: B,N,F,H = 8,1024,1024,8 (HD=128). Data-parallel over B across the
8 NeuronCores (one batch element per core, weights replicated; no
collectives). Per core:

    q = obj @ Wq.T * 1/sqrt(HD)   (scale folded into Wq host-side)
    k = cross @ Wk.T
    vW = cross @ Wvo + bo'        (host-fused Wvo = Wv.T @ WoT, so the
                                   v-projection and the v@Wo.T reduction
                                   collapse into ONE matmul; bo' absorbs
                                   bv@WoT + bo, valid because softmax rows
                                   sum to 1)
    att_h = q_h @ k_h.T + M       (M = label_bias + (adj-1)*9e15, injected
                                   into PSUM by an identity-stationary
                                   matmul -- no elementwise mask pass)
    A_u_h = exp(att_h)            (masked entries underflow to exact 0)
    S_h   = rowsum(A_u_h)         (free via the Exp activation's accum_out)
    out_h = (A_u_h @ vW_h) / S_h  (normalization deferred past the AV
                                   matmul, applied as a per-partition scalar)
    att_avg = sum_h A_u_h / (S_h * H)

All matmuls run in bf16 (fp32 PSUM accumulation). The softmax skips rowmax
subtraction: scores are ~N(0, 0.41) so exp() is safely in range. All layout
transposes (obj/cross feature-major, A_u -> A_u^T for the AV contraction,
per-head output back to row-major) ride the DMA XBAR transpose engine
(2-byte dtype) instead of the TensorEngine. Emission interleaves the
projection matmuls with the per-head attention pipeline so softmax ACT/DVE
work hides under projection PE work.
"""

import sys

sys.path.insert(0, "/opt/trn_rl_repo")

import contextlib

import numpy as np
import ml_dtypes

import concourse.bass as bass
import concourse.tile as tile
from concourse import bacc, mybir
from concourse.bass_utils import run_bass_kernel_spmd
from concourse.masks import make_identity

BF16 = mybir.dt.bfloat16
F32 = mybir.dt.float32
AF = mybir.ActivationFunctionType
ALU = mybir.AluOpType

P = 128
B, N, F, H = 8, 1024, 1024, 8
HD = F // H  # 128
CH = F // P  # 8 feature chunks
NCH = N // P  # 8 row chunks
NH = N // 512  # 2 free-dim halves

_PROG = None  # cached compiled Bass program


def _build_program(time_reps=1, with_bias=True):
    """time_reps>1 wraps the body in a hardware loop so marginal wall-clock
    per iteration isolates true NEFF execution time from the remote-dispatch
    floor. with_bias=False drops the per-partition q/k bias adds (all-zero
    biases) so projection PSUM->SBUF copies can balance across engines."""
    nc = bacc.Bacc("TRN2", target_bir_lowering=False, debug=False, num_devices=8)

    obj_d = nc.dram_tensor("obj", [N, F], BF16, kind="ExternalInput")
    cross_d = nc.dram_tensor("cross", [N, F], BF16, kind="ExternalInput")
    mcomb_d = nc.dram_tensor("mcomb", [N, N], BF16, kind="ExternalInput")
    wqt_d = nc.dram_tensor("wqt", [F, F], BF16, kind="ExternalInput")
    wkt_d = nc.dram_tensor("wkt", [F, F], BF16, kind="ExternalInput")
    wvo_d = nc.dram_tensor("wvo", [F, F], BF16, kind="ExternalInput")
    bq_d = nc.dram_tensor("bq", [F], F32, kind="ExternalInput")
    bk_d = nc.dram_tensor("bk", [F], F32, kind="ExternalInput")
    bo_rep_d = nc.dram_tensor("bo_rep", [P, F], BF16, kind="ExternalInput")
    out_d = nc.dram_tensor("out", [N, F], F32, kind="ExternalOutput")
    avg_d = nc.dram_tensor("att_avg", [N, N], F32, kind="ExternalOutput")

    with tile.TileContext(nc) as tc:
        with (
            tc.For_i(0, time_reps, 1) if time_reps > 1 else contextlib.nullcontext(),
            tc.tile_pool(name="persist", bufs=1) as persist,
            tc.tile_pool(name="wpool", bufs=1) as wpool,
            tc.tile_pool(name="big", bufs=4) as big,
            tc.tile_pool(name="stage", bufs=2) as stage,
            tc.tile_pool(name="small", bufs=3) as small,
            tc.tile_pool(name="psA", bufs=2, space="PSUM") as psA,
            tc.tile_pool(name="psatt", bufs=2, space="PSUM") as psatt,
            tc.tile_pool(name="psav", bufs=2, space="PSUM") as psav,
        ):
            qT = persist.tile([P, CH, N], BF16, tag="qT")
            kT = persist.tile([P, CH, N], BF16, tag="kT")
            vW = persist.tile([P, CH, F], BF16, tag="vW")
            mcomb = persist.tile([P, NCH, N], BF16, tag="mcomb")
            acc = persist.tile([P, NCH, N], BF16, tag="acc")
            bo_rep = persist.tile([P, F], BF16, tag="bo_rep")
            ident = persist.tile([P, P], BF16, tag="ident")
            make_identity(nc, ident[:])

            nc.sync.dma_start(bo_rep[:], bo_rep_d[:])
            nc.sync.dma_start(
                mcomb[:], mcomb_d.ap().rearrange("(no p) m -> p no m", p=P)
            )
            if with_bias:
                bq_t = persist.tile([P, CH], F32, tag="bq")
                bk_t = persist.tile([P, CH], F32, tag="bk")
                nc.sync.dma_start(bq_t[:], bq_d.ap().rearrange("(o p) -> p o", p=P))
                nc.sync.dma_start(bk_t[:], bk_d.ap().rearrange("(o p) -> p o", p=P))

            def transpose_in(x_dram):
                """[N, F] bf16 DRAM -> [P, CH, N] bf16 SBUF feature-major via
                DMA XBAR transpose."""
                xT = big.tile([P, CH, N], BF16, tag="big")
                for no in range(NCH):
                    nc.sync.dma_start_transpose(
                        xT[:, :, no * P : (no + 1) * P],
                        x_dram.ap()[no * P : (no + 1) * P, :],
                    )
                return xT

            def project_chunk(dst, wT, srcT, fo, bias_t):
                """dst[:, fo*P:] = (srcT.T @ wT[:, :, fo])^T for one output
                feature chunk fo (16 matmuls, accumulate over CH)."""
                for nh in range(NH):
                    ps = psA.tile([P, 512], F32, tag="psA")
                    for co in range(CH):
                        nc.tensor.matmul(
                            ps[:],
                            lhsT=wT[:, co, fo * P : (fo + 1) * P],
                            rhs=srcT[:, co, nh * 512 : (nh + 1) * 512],
                            start=(co == 0),
                            stop=(co == CH - 1),
                        )
                    dslc = dst[:, fo, nh * 512 : (nh + 1) * 512]
                    if with_bias:
                        nc.scalar.activation(
                            dslc, ps[:], AF.Identity, bias=bias_t[:, fo : fo + 1]
                        )
                    else:
                        nc.any.tensor_copy(dslc, ps[:])

            st = {}  # per-head stage-1 products

            def stage1(h):
                A_u = big.tile([P, NCH, N], BF16, tag="big")
                S = small.tile([P, NCH], F32, tag="S")
                for no in range(NCH):
                    pa = psatt.tile([P, N], F32, tag="att")
                    for mh in range(NH):
                        nc.tensor.matmul(
                            pa[:, mh * 512 : (mh + 1) * 512],
                            lhsT=qT[:, h, no * P : (no + 1) * P],
                            rhs=kT[:, h, mh * 512 : (mh + 1) * 512],
                            start=True,
                            stop=False,
                        )
                        # additive mask via identity-stationary matmul:
                        # psum += I.T @ mcomb = mcomb
                        nc.tensor.matmul(
                            pa[:, mh * 512 : (mh + 1) * 512],
                            lhsT=ident[:],
                            rhs=mcomb[:, no, mh * 512 : (mh + 1) * 512],
                            start=False,
                            stop=True,
                        )
                    # masked exp + row sums in one ACT pass
                    nc.scalar.activation(
                        A_u[:, no, :], pa[:], AF.Exp, accum_out=S[:, no : no + 1]
                    )
                rs = small.tile([P, NCH], F32, tag="rs")
                rs8 = small.tile([P, NCH], F32, tag="rs8")
                nc.vector.reciprocal(rs[:], S[:])
                nc.vector.tensor_scalar_mul(rs8[:], rs[:], 1.0 / H)
                st[h] = (A_u, rs, rs8)

            def stage2(h):
                A_u, rs, rs8 = st.pop(h)
                # transpose A_u via DMA XBAR: A_uT[p,mo,n] = A_u[n, mo*128+p]
                A_uT = big.tile([P, CH, N], BF16, tag="big")
                for no in range(NCH):
                    nc.sync.dma_start_transpose(
                        A_uT[:, :, no * P : (no + 1) * P], A_u[:, no, :]
                    )
                # outT[hd, n] = sum_m vW[m, h*HD+hd] * A_uT[m, n]
                outT = stage.tile([P, N], BF16, tag="outT")
                for ng in range(NH):
                    pav = psav.tile([P, 512], F32, tag="av")
                    for mo in range(CH):
                        nc.tensor.matmul(
                            pav[:],
                            lhsT=vW[:, mo, h * HD : (h + 1) * HD],
                            rhs=A_uT[:, mo, ng * 512 : (ng + 1) * 512],
                            start=(mo == 0),
                            stop=(mo == CH - 1),
                        )
                    nc.any.tensor_copy(outT[:, ng * 512 : (ng + 1) * 512], pav[:])
                # back to row-major: outN[p, no, hd] = outT[hd, no*128+p]
                outN = stage.tile([P, NCH, HD], BF16, tag="outN")
                nc.sync.dma_start_transpose(outN[:], outT[:])
                for no in range(NCH):
                    ot = small.tile([P, HD], F32, tag="ot")
                    nc.vector.tensor_scalar_mul(
                        ot[:], outN[:, no, :], rs[:, no : no + 1]
                    )
                    nc.sync.dma_start(
                        out_d.ap()[no * P : (no + 1) * P, h * HD : (h + 1) * HD],
                        ot[:],
                    )
                # att_avg accumulation
                for no in range(NCH):
                    if h == 0:
                        nc.vector.tensor_scalar_mul(
                            acc[:, no, :], A_u[:, no, :], rs8[:, no : no + 1]
                        )
                    else:
                        nc.vector.scalar_tensor_tensor(
                            out=acc[:, no, :],
                            in0=A_u[:, no, :],
                            scalar=rs8[:, no : no + 1],
                            in1=acc[:, no, :],
                            op0=ALU.mult,
                            op1=ALU.add,
                        )

            # ---- emission: vW + kT early (frees crossT), then per-head
            # pipeline interleaved with the q projections ----
            crossT = transpose_in(cross_d)
            wvo = big.tile([P, CH, F], BF16, tag="big")
            nc.sync.dma_start(wvo[:], wvo_d.ap().rearrange("(co p) f -> p co f", p=P))
            for mo in range(CH):
                for fh in range(NH):
                    ps = psA.tile([P, 512], F32, tag="psA")
                    for co in range(CH):
                        nc.tensor.matmul(
                            ps[:],
                            lhsT=crossT[:, co, mo * P : (mo + 1) * P],
                            rhs=wvo[:, co, fh * 512 : (fh + 1) * 512],
                            start=(co == 0),
                            stop=(co == CH - 1),
                        )
                    nc.vector.tensor_add(
                        vW[:, mo, fh * 512 : (fh + 1) * 512],
                        ps[:],
                        bo_rep[:, fh * 512 : (fh + 1) * 512],
                    )

            wk = wpool.tile([P, CH, F], BF16, tag="wk")
            nc.sync.dma_start(wk[:], wkt_d.ap().rearrange("(co p) f -> p co f", p=P))
            for fo in range(CH):
                project_chunk(kT, wk, crossT, fo, bk_t if with_bias else None)

            wq = wpool.tile([P, CH, F], BF16, tag="wq")
            nc.sync.dma_start(wq[:], wqt_d.ap().rearrange("(co p) f -> p co f", p=P))
            objT = transpose_in(obj_d)
            for fo in range(CH):
                project_chunk(qT, wq, objT, fo, bq_t if with_bias else None)
                stage1(fo)
                if fo > 0:
                    stage2(fo - 1)
            stage2(H - 1)

            # ---- att_avg convert + out ----
            for no in range(NCH):
                cv = stage.tile([P, N], F32, tag="cvf")
                nc.gpsimd.tensor_copy(cv[:], acc[:, no, :])
                nc.sync.dma_start(avg_d.ap()[no * P : (no + 1) * P, :], cv[:])

    nc.compile()
    return nc


def _get_program(with_bias=True):
    global _PROG
    if _PROG is None or _PROG[1] != with_bias:
        _PROG = (_build_program(with_bias=with_bias), with_bias)
    return _PROG[0]


def _prep_inputs(
    obj_feats, cross_feats, adj_matrix, label_biases_att,
    Wq, bq, Wk, bk, Wv, bv, Wo, bo,
):
    bf16 = ml_dtypes.bfloat16
    s = np.float32(1.0 / np.sqrt(HD))
    wqt = np.ascontiguousarray((Wq.T * s).astype(bf16))  # [C, F], scale folded
    wkt = np.ascontiguousarray(Wk.T.astype(bf16))
    # WoT[f, h*HD+hd] = Wo[h, hd, f]; Wvo = Wv.T @ WoT fuses v-proj with v@Wo.T
    wot = Wo.transpose(2, 0, 1).reshape(F, F)
    wvo = np.ascontiguousarray((Wv.T @ wot).astype(bf16))
    # bo' = bo + bv @ WoT (valid since softmax rows sum to 1)
    bo_eff = bo + bv @ wot
    bo_rep = np.ascontiguousarray(np.broadcast_to(bo_eff, (P, F)).astype(bf16))
    bq_s = (bq * s).astype(np.float32)
    # additive mask: label_bias where adj>0 else -9e15 (exp underflows to 0)
    mcomb = np.where(
        adj_matrix > 0, label_biases_att, np.float32(-9e15) + label_biases_att
    ).astype(bf16)
    obj16 = obj_feats.astype(bf16)
    cross16 = cross_feats.astype(bf16)

    with_bias = bool(np.any(bq) or np.any(bk))
    in_maps = []
    for b in range(B):
        in_maps.append(
            {
                "obj": np.ascontiguousarray(obj16[b]),
                "cross": np.ascontiguousarray(cross16[b]),
                "mcomb": np.ascontiguousarray(mcomb[b]),
                "wqt": wqt,
                "wkt": wkt,
                "wvo": wvo,
                "bq": bq_s,
                "bk": bk.astype(np.float32),
                "bo_rep": bo_rep,
            }
        )
    return in_maps, with_bias


def kernel(
    obj_feats, cross_feats, adj_matrix, label_biases_att,
    Wq, bq, Wk, bk, Wv, bv, Wo, bo,
):
    args = [
        np.asarray(obj_feats, np.float32),
        np.asarray(cross_feats, np.float32),
        np.asarray(adj_matrix),
        np.asarray(label_biases_att, np.float32),
        np.asarray(Wq, np.float32),
        np.asarray(bq, np.float32),
        np.asarray(Wk, np.float32),
        np.asarray(bk, np.float32),
        np.asarray(Wv, np.float32),
        np.asarray(bv, np.float32),
        np.asarray(Wo, np.float32),
        np.asarray(bo, np.float32),
    ]
    in_maps, with_bias = _prep_inputs(*args)
    nc = _get_program(with_bias=with_bias)
    res = run_bass_kernel_spmd(nc, in_maps, core_ids=list(range(B)))
    out = np.stack([res.results[b]["out"] for b in range(B)])
    att_avg = np.stack([res.results[b]["att_avg"] for b in range(B)])
    return out, att_avg
